# revision 1
# baseline (speedup 1.0000x reference)
"""BinNorm (sum-of-sigmoids row normalization via root-find) for Trainium2.

Math: for each row x of shape [256], find nu s.t. sum(sigmoid(x + nu)) == 64,
then output sigmoid(x + nu).  The reference finds nu by a branch-lattice
bisection whose final bracket width is ~6.8e-5 (it quantizes nu to the bracket
midpoint).  Any nu within that quantization radius of the true root produces
outputs within ~1e-5 absmax of the reference, below the fp32 reordering noise
floor of the reference itself (~1.7e-5).

Kernel algorithm per row:
  1. mean/var via bn_stats -> quadratic-poly initializer nu0 (max err ~0.03)
  2. Newton step   (sigmoid ACT pass with row-accumulate f; DVE sum sigma^2)
  3. chord step    (one more sigmoid pass, reuse the Newton reciprocal slope)
  4. output pass   sigmoid(x + nu2), batched per store block: x+nu2 pre-added
     on the idle GPSIMD engine, one wide sigmoid on ACT
Eval sigmoids are single ACT instructions over [128, 256] tiles using the
per-partition bias + accum_out features.

Sharding: pure data parallel over rows, 8 cores x 2048 rows.
"""

import os as _os
import numpy as np

_CORES = 8
_B, _D = 16384, 256
_BC = _B // _CORES          # rows per core
_P = 128                    # partitions
_T = _BC // _P              # 16 row-tiles per core

# per-group tile counts (first groups small to shorten the startup chain)
_GROUPS = tuple(int(v) for v in _os.environ.get(
    "BK_GROUPS", "1,1,1,1,2,2,2,2,1,1,1,1").split(","))
_SCR_BUFS = int(_os.environ.get("BK_SCR_BUFS", "16"))
# input/output DMA block sizes (in 128-row tiles); loads front-loaded small,
# stores tail-loaded small.  width>=2 out blocks get a batched output pass.
_IN_BLOCKS = tuple(int(v) for v in _os.environ.get(
    "BK_IN_BLOCKS", "1,1,2,2,2,4,2,2").split(","))
_OUT_BLOCKS = tuple(int(v) for v in _os.environ.get(
    "BK_OUT_BLOCKS", "4,2,2,2,2,2,1,1").split(","))
_PRE_ENG = _os.environ.get("BK_PRE_ENG", "gpsimd")  # engine for x+nu pre-adds
_SCHEME = _os.environ.get("BK_SCHEME", "newton2")     # halley | newton2
_CU_ENG = _os.environ.get("BK_CU_ENG", "vector")     # engine for sigma^3
_BN_GROUP = _os.environ.get("BK_BN_GROUP", "0") == "1"
_LOOKAHEAD = int(_os.environ.get("BK_LOOKAHEAD", "2"))
_POLY_GP = _os.environ.get("BK_POLY_GP", "0") == "1"
_SW_LOADS = int(_os.environ.get("BK_SW_LOADS", "0"))
_ACT_STORES = int(_os.environ.get("BK_ACT_STORES", "0"))
_HALLEY_SET = set(int(v) for v in _os.environ.get("BK_HALLEY_SET", "1,3,5,7,8,9,10,11").split(",") if v)

# nu0 = C0 + C1*m + C2*v + C3*m^2 + C4*m*v + C5*v^2  (m=row mean, v=row var),
# least-squares fit of the true root over N(0,1) rows.
_C = (-1.097386107696299, -1.0174597913968035, -0.24531199751746788,
      0.010321566224828467, 0.005161273657493432, 0.027572120704527067)

_KF = 64.0                  # target sum

_cache: dict = {}


def _build_nc():
    from contextlib import ExitStack
    import concourse.bacc as bacc
    import concourse.mybir as mybir
    import concourse.tile as tile

    f32 = mybir.dt.float32
    SIG = mybir.ActivationFunctionType.Sigmoid
    A = mybir.AluOpType

    assert sum(_IN_BLOCKS) == _T and sum(_OUT_BLOCKS) == _T
    assert sum(_GROUPS) == _T

    nc = bacc.Bacc(
        "TRN2",
        target_bir_lowering=False,
        debug=False,
        enable_asserts=False,
        num_devices=_CORES,
    )
    x = nc.dram_tensor("x", [_BC, _D], f32, kind="ExternalInput").ap()
    y = nc.dram_tensor("y", [_BC, _D], f32, kind="ExternalOutput").ap()

    with tile.TileContext(nc) as tc, ExitStack() as ctx:
        xp = ctx.enter_context(tc.tile_pool(name="xp", bufs=1))
        sp = ctx.enter_context(tc.tile_pool(name="sp", bufs=_SCR_BUFS))
        op = ctx.enter_context(tc.tile_pool(name="op", bufs=1))
        st = ctx.enter_context(tc.tile_pool(name="st", bufs=1))

        pre_eng = nc.gpsimd if _PRE_ENG == "gpsimd" else nc.vector
        cu_eng = nc.gpsimd if _CU_ENG == "gpsimd" else nc.vector

        # warmup: trigger the sigmoid table load before any data arrives
        wz = st.tile([_P, 1], f32, tag="wz", name="wz")
        nc.vector.memset(wz[:], 0.0)
        wo = st.tile([_P, 1], f32, tag="wo", name="wo")
        nc.scalar.activation(wo[:], wz[:], SIG, bias=wz[:])

        # blocked loads: xt[t] are column views into the block tiles
        xt = [None] * _T
        xwhere = [None] * _T
        t = 0
        for b, w in enumerate(_IN_BLOCKS):
            blk = xp.tile([_P, w * _D], f32, tag=f"xb{b}", name=f"xb{b}")
            src = x[t * _P:(t + w) * _P, :].rearrange("(t p) d -> p t d", p=_P)
            ldeng = nc.gpsimd if b < _SW_LOADS else nc.sync
            ldeng.dma_start(blk[:].rearrange("p (t d) -> p t d", d=_D), src)
            for j in range(w):
                xt[t + j] = blk[:, (j * _D):(j + 1) * _D]
                xwhere[t + j] = (blk, j)
            t += w

        # out block tiles; a block's output pass is emitted once every tile's
        # nu2 is known (nu2col[t] below)
        oblk = []           # [blk, t0, w]
        t = 0
        for b, w in enumerate(_OUT_BLOCKS):
            blk = op.tile([_P, w * _D], f32, tag=f"ob{b}", name=f"ob{b}")
            oblk.append([blk, t, w])
            t += w

        nu2col = [None] * _T      # per-tile [P,1] view of its group's nu2

        def emit_ready_outputs():
            while oblk and all(nu2col[t] is not None
                               for t in range(oblk[0][1],
                                              oblk[0][1] + oblk[0][2])):
                blk, t0, w = oblk.pop(0)
                if w >= 2:
                    pre = sp.tile([_P, w * _D], f32, tag="pre",
                                  name=f"pre_{t0}")
                    for j in range(w):
                        pre_eng.tensor_scalar_add(
                            pre[:, j * _D:(j + 1) * _D], xt[t0 + j],
                            nu2col[t0 + j])
                    nc.scalar.activation(blk[:], pre[:], SIG)
                else:
                    for j in range(w):
                        nc.scalar.activation(
                            blk[:, j * _D:(j + 1) * _D], xt[t0 + j], SIG,
                            bias=nu2col[t0 + j])
                dst = y[t0 * _P:(t0 + w) * _P, :].rearrange(
                    "(t p) d -> p t d", p=_P)
                steng = nc.scalar if (t0 + w > _T - _ACT_STORES) else nc.sync
                steng.dma_start(dst, blk[:].rearrange("p (t d) -> p t d",
                                                      d=_D))

        group_t0 = []
        _acc = 0
        for G in _GROUPS:
            group_t0.append(_acc)
            _acc += G

        nu0_of = {}

        def emit_init(g):
            G = _GROUPS[g]
            t0 = group_t0[g]

            def stile(tag, w=G):
                return st.tile([_P, w], f32, tag=tag, name=tag)

            # ---- moments ----
            agg = st.tile([_P, 2 * G], f32, tag=f"agg{g}", name=f"agg{g}")
            aggv = agg[:].rearrange("p (c g) -> p c g", g=G)  # [P,2,G]
            xb0, xc0 = xwhere[t0]
            xbN, xcN = xwhere[t0 + G - 1]
            if _BN_GROUP and G >= 2 and xb0 is xbN and xcN == xc0 + G - 1:
                bn6 = st.tile([_P, 6 * G], f32, tag=f"bn6_{g}",
                              name=f"bn6_{g}")
                src3 = xb0[:, xc0 * _D:(xc0 + G) * _D].rearrange(
                    "p (t d) -> p t d", d=_D)
                nc.vector.bn_stats(
                    bn6[:].rearrange("p (t c) -> p t c", c=6), src3)
                bn6v = bn6[:].rearrange("p (t c) -> p t c", c=6)
                for j in range(G):
                    nc.vector.bn_aggr(aggv[:, :, j], bn6v[:, j, :])
            else:
                for j in range(G):
                    bn6 = st.tile([_P, 6], f32, tag=f"bn6_{g}_{j}",
                                  name=f"bn6_{g}_{j}")
                    nc.vector.bn_stats(bn6[:], xt[t0 + j])
                    nc.vector.bn_aggr(aggv[:, :, j], bn6[:])
            m1 = aggv[:, 0, :]   # [P,G] mean
            vv = aggv[:, 1, :]   # [P,G] var

            # ---- initializer poly (dep depth 4) ----
            peng = pre_eng if _POLY_GP else nc.vector
            t1 = stile(f"t1_{g}")
            peng.tensor_scalar(t1[:], m1, _C[3], _C[1], A.mult, A.add)
            t4 = stile(f"t4_{g}")
            peng.tensor_scalar(t4[:], vv, _C[5], _C[2], A.mult, A.add)
            t2 = stile(f"t2_{g}")
            nc.vector.scalar_tensor_tensor(t2[:], vv, _C[4], t1[:], A.mult, A.add)
            t5 = stile(f"t5_{g}")
            nc.vector.tensor_mul(t5[:], t4[:], vv)
            t3 = stile(f"t3_{g}")
            nc.vector.tensor_mul(t3[:], t2[:], m1)
            nu0 = stile(f"nu0_{g}")
            nc.vector.scalar_tensor_tensor(nu0[:], t3[:], _C[0], t5[:],
                                           A.add, A.add)

            nu0_of[g] = nu0

        def emit_compute(g):
            G = _GROUPS[g]
            t0 = group_t0[g]
            nu0 = nu0_of[g]

            def stile(tag, w=G):
                return st.tile([_P, w], f32, tag=tag, name=tag)

            if _SCHEME == "halley" or g in _HALLEY_SET:
                # ---- single eval pass: S1=sum s, S2=sum s^2, S3=sum s^3 ----
                S1 = stile(f"S1_{g}")
                S2 = stile(f"S2_{g}")
                S3 = stile(f"S3_{g}")
                for j in range(G):
                    scr = sp.tile([_P, _D], f32, tag="scr", name=f"scr_{g}_{j}")
                    nc.scalar.activation(scr[:], xt[t0 + j], SIG,
                                         bias=nu0[:, j:j + 1],
                                         accum_out=S1[:, j:j + 1])
                    sq = sp.tile([_P, _D], f32, tag="sq", name=f"sq_{g}_{j}")
                    nc.vector.scalar_tensor_tensor(
                        sq[:], scr[:], 0.0, scr[:], A.add, A.mult,
                        accum_out=S2[:, j:j + 1])
                    cu = sp.tile([_P, _D], f32, tag="cu", name=f"cu_{g}_{j}")
                    cu_eng.scalar_tensor_tensor(
                        cu[:], sq[:], 0.0, scr[:], A.add, A.mult,
                        accum_out=S3[:, j:j + 1])
                # ---- Halley: nu2 = nu0 - f*fp / (fp^2 - f*fpp/2) ----
                fp = stile(f"fp_{g}")
                nc.vector.tensor_sub(fp[:], S1[:], S2[:])
                u6 = stile(f"u6_{g}")
                nc.vector.scalar_tensor_tensor(u6[:], S2[:], -3.0, S1[:],
                                               A.mult, A.add)
                fpp = stile(f"fpp_{g}")
                nc.vector.scalar_tensor_tensor(fpp[:], S3[:], 2.0, u6[:],
                                               A.mult, A.add)
                n1 = stile(f"n1_{g}")
                nc.vector.scalar_tensor_tensor(n1[:], S1[:], -_KF, fp[:],
                                               A.add, A.mult)
                d1 = stile(f"d1_{g}")
                nc.vector.tensor_mul(d1[:], fp[:], fp[:])
                d2 = stile(f"d2_{g}")
                nc.vector.scalar_tensor_tensor(d2[:], S1[:], -_KF, fpp[:],
                                               A.add, A.mult)
                den = stile(f"den_{g}")
                nc.vector.scalar_tensor_tensor(den[:], d2[:], -0.5, d1[:],
                                               A.mult, A.add)
                rec = stile(f"rec_{g}")
                nc.vector.reciprocal(rec[:], den[:])
                stp = stile(f"stp_{g}")
                nc.vector.tensor_mul(stp[:], n1[:], rec[:])
                nu2 = stile(f"nu2_{g}")
                nc.vector.tensor_sub(nu2[:], nu0[:], stp[:])
            else:
                # ---- Newton step: nu1 = nu0 - (f0-K)/(f0-q0) ----
                f0 = stile(f"f0_{g}")
                q0 = stile(f"q0_{g}")
                for j in range(G):
                    scr = sp.tile([_P, _D], f32, tag="scr", name=f"scr_{g}_{j}")
                    nc.scalar.activation(scr[:], xt[t0 + j], SIG,
                                         bias=nu0[:, j:j + 1],
                                         accum_out=f0[:, j:j + 1])
                    sq = sp.tile([_P, _D], f32, tag="sq", name=f"sq_{g}_{j}")
                    nc.vector.scalar_tensor_tensor(
                        sq[:], scr[:], 0.0, scr[:], A.add, A.mult,
                        accum_out=q0[:, j:j + 1])
                fp = stile(f"fp_{g}")
                nc.vector.tensor_sub(fp[:], f0[:], q0[:])
                rp = stile(f"rp_{g}")
                nc.vector.reciprocal(rp[:], fp[:])
                stp = stile(f"stp_{g}")
                nc.vector.scalar_tensor_tensor(stp[:], f0[:], -_KF, rp[:],
                                               A.add, A.mult)
                nu1 = stile(f"nu1_{g}")
                nc.vector.tensor_sub(nu1[:], nu0[:], stp[:])

                # ---- chord step: nu2 = nu1 - (f1-K)*rp ----
                f1 = stile(f"f1_{g}")
                for j in range(G):
                    scr3 = sp.tile([_P, _D], f32, tag="scr3",
                                   name=f"scr3_{g}_{j}")
                    nc.scalar.activation(scr3[:], xt[t0 + j], SIG,
                                         bias=nu1[:, j:j + 1],
                                         accum_out=f1[:, j:j + 1])
                stp1 = stile(f"stp1_{g}")
                nc.vector.scalar_tensor_tensor(stp1[:], f1[:], -_KF, rp[:],
                                               A.add, A.mult)
                nu2 = stile(f"nu2_{g}")
                nc.vector.tensor_sub(nu2[:], nu1[:], stp1[:])

            for j in range(G):
                nu2col[t0 + j] = nu2[:, j:j + 1]
            emit_ready_outputs()


        for g in range(min(_LOOKAHEAD, len(_GROUPS))):
            emit_init(g)
        for g in range(len(_GROUPS)):
            la = g + _LOOKAHEAD
            if la < len(_GROUPS):
                emit_init(la)
            emit_compute(g)
        assert not oblk

    nc.compile()
    return nc


def _get_nc():
    if "nc" not in _cache:
        _cache["nc"] = _build_nc()
    return _cache["nc"]


def kernel(x: np.ndarray) -> np.ndarray:
    from concourse.bass_utils import run_bass_kernel_spmd

    x = np.ascontiguousarray(x, dtype=np.float32)
    assert x.shape == (_B, _D), x.shape

    nc = _get_nc()
    in_maps = [{"x": x[i * _BC:(i + 1) * _BC]} for i in range(_CORES)]
    res = run_bass_kernel_spmd(nc, in_maps, list(range(_CORES)))
    out = np.concatenate([res.results[i]["y"] for i in range(_CORES)], axis=0)
    return out.astype(np.float32)

